# revision 2
# baseline (speedup 1.0000x reference)
"""Trainium2 Bass kernel for nn_DivEncLayer (grouped per-slice MLP 8->32->1).

Reference (per batch row b, slice q of 128):
    xs = x.reshape(B, 128, 8)
    h  = ELU(xs[b,q,:] @ W1[q] + b1[q])
    h  = (h - mov_mean[q]) * gamma[q]/sqrt(mov_var[q]+eps) + beta[q]
    out[b,q] = h @ W2[q] + b2[q]

v3 design (pure data parallel over 8 NeuronCores, B=32768 -> 4096/core):
  * BN folds into w2p = gamma/sqrt(var+eps)*W2; constant bfin.
  * key identity:  ELU(u)+1 = u - v + exp(v),  v = min(u,0)
      - the u term is LINEAR in x -> folded on host into weff = sum_h w2p*W1
        and streamed as a tiny extra matmul (lin)
      - v needs ONE elementwise PSUM pass (DVE tensor_scalar)
      - T = exp(v) reads v from SBUF -> full-width ACT instrs (64/core)
    so per u element: 1 DVE op + 1 ACT op (v2 needed 3 DVE-class ops).
  * dense2: out_q = lin - sum_h w2p*v + sum_h w2p*T + bfin
    per group-pair PSUM chain: 2 lin mms + 4 (-w2p)-v mms x2 + 4 w2p-T mms x2.
  * bias+copy of outq via ACT Identity(bias=bfin) -> SBUF -> DMA.
  * x cast to fp16 on host; loaded transposed from HBM by XBAR DMA (16x128
    tiles): no PE transposes, no PSUM->SBUF x copies.
  * everything 16-bit is fp16 (not bf16): l2 ~5e-4 vs reference.

HW constraints handled:
  * one semaphore wait per instruction (walrus)        -> _split_waits
  * PSUM matmul outputs bank-aligned (2KB zero region) -> NB=512
  * accumulation chain shares tile_position; base partition 32-aligned
"""

import sys

for _p in ("/opt/trn_rl_repo", "/root/.axon_site/_ro/trn_rl_repo"):
    if _p not in sys.path:
        sys.path.append(_p)

import contextlib
import os as _os

import numpy as np

import concourse.bass as bass
import concourse.tile as tile
from concourse import mybir
from concourse.bass_utils import run_bass_kernel_spmd

F32 = mybir.dt.float32
F16 = mybir.dt.float16

Q, S, H = 128, 8, 32
C = Q * S
NCORES = 8
BN_EPS = 1e-3
NB = 512                       # batch tile; one PSUM bank per [128, NB] f32
NG = 8                         # c-groups of 16 slices

# engine for the v = min(u,0) pass, per group: 'v' = DVE, 'p' = GpSimd
VMIN_ENG = _os.environ.get("VMIN_ENG", "vvvvvvvv")
# engine for the outq bias+copy: 'a' = ACT, 'v' = DVE, 'p' = GpSimd
OUT_ENG = _os.environ.get("OUT_ENG", "a")

_NOPN = [0]


def _split_waits(tc):
    """walrus supports only one sync-wait command per instruction; Tile can
    emit several.  Precede every multi-wait instruction with same-engine
    NoOps carrying all but the last wait."""
    orig = tc._add_instruction

    def patched(inst):
        si = inst.sync_info
        if (
            not inst.name.startswith("waitnop")
            and si is not None
            and len(si.on_wait) > 1
        ):
            for w in si.on_wait[:-1]:
                _NOPN[0] += 1
                nop = mybir.InstNoOp(name=f"waitnop-{_NOPN[0]}", ins=[], outs=[])
                nop.engine = inst.engine
                nop.sync_info = mybir.SyncInfo(on_wait=[w], on_update=[])
                orig(nop)
            inst.sync_info = mybir.SyncInfo(
                on_wait=[si.on_wait[-1]], on_update=list(si.on_update)
            )
        return orig(inst)

    tc._add_instruction = patched

    def patched_dab(tick_clock, wait_clock):
        from concourse.vector_clock import ScopedClock

        nc = tc.nc
        drain_inst = nc.sync.drain()
        wait_clock.add_sem_waits(
            drain_inst.ins, ScopedClock({None: tick_clock.global_clock})
        )
        si = drain_inst.ins.sync_info
        if si is not None and len(si.on_wait) > 1:
            extra = list(si.on_wait[1:])
            drain_inst.ins.sync_info = mybir.SyncInfo(
                on_wait=[si.on_wait[0]], on_update=list(si.on_update)
            )
            for w in extra:
                n = nc.sync.nop(nofuse=True)
                n.ins.sync_info = mybir.SyncInfo(on_wait=[w], on_update=[])

        nc.all_engine_barrier()
        assert tc.sems is not None
        popped = nc._tile_sem_poison_stack.pop()
        assert popped is tc._sem_poison
        nc.clear_and_free_semaphores(list(tc.sems.allocated().values()))
        nc.all_engine_barrier()

    tc._drain_and_barrier = patched_dab


def _host_pack(W1, b1, gamma, beta, mov_mean, mov_var, W2, b2):
    """Fold BN into dense2; pack block weights for the PE layouts."""
    W1 = np.asarray(W1, np.float32).reshape(Q, S, H)
    b1 = np.asarray(b1, np.float32).reshape(Q, H)
    gamma = np.asarray(gamma, np.float32).reshape(Q, H)
    beta = np.asarray(beta, np.float32).reshape(Q, H)
    mean = np.asarray(mov_mean, np.float32).reshape(Q, H)
    var = np.asarray(mov_var, np.float32).reshape(Q, H)
    W2 = np.asarray(W2, np.float32).reshape(Q, H)
    b2 = np.asarray(b2, np.float32).reshape(Q)

    f16 = np.float16

    inv = gamma / np.sqrt(var + BN_EPS)
    w2p = (inv * W2).astype(np.float32)                      # [Q,H]
    # out = lin - sum w2p*v + sum w2p*T + bfin
    bfin = (b2 + ((beta - mean * inv) * W2).sum(-1) - w2p.sum(-1)).astype(np.float32)

    W1q = W1.astype(f16).astype(np.float32)   # device-visible W1

    # dense1 stationary: MM (g,i) is a K=128 matmul, block-diagonal lhsT
    # (rows 32i..32i+32 live) computing slices q=16g+4i+j at partitions 32j+h.
    w1bd = np.zeros((128, NG, 4, 128), np.float32)
    for g in range(NG):
        for i in range(4):
            for j in range(4):
                q = 16 * g + 4 * i + j
                w1bd[32 * i + 8 * j:32 * i + 8 * j + 8, g, i, 32 * j:32 * j + 32] = W1q[q]

    # dense2 stationary (T stream, +w2p): for (g,i), col m = 16*(g%2)+4i+j
    # holds w2p of slice q = 16g+4i+j at rows 32j..32j+32.
    w2m = np.zeros((128, NG, 4, 32), np.float32)
    for g in range(NG):
        for i in range(4):
            for j in range(4):
                q = 16 * g + 4 * i + j
                m = 16 * (g % 2) + 4 * i + j
                w2m[32 * j:32 * j + 32, g, i, m] = w2p[q]

    # lin stationary: for group g, col m = 16*(g%2)+idx holds
    # weff[q=16g+idx, s] at rows 8*idx..8*idx+8 (xt partition = 8*idx+s).
    weff = np.einsum('qsh,qh->qs', W1q, w2p)                 # fold in f32
    wlin = np.zeros((128, NG, 32), np.float32)
    for g in range(NG):
        for idx in range(16):
            q = 16 * g + idx
            m = 16 * (g % 2) + idx
            wlin[8 * idx:8 * idx + 8, g, m] = weff[q]

    has_b1 = bool(np.any(b1 != 0.0))
    # per-partition b1 for the (rare) b1 != 0 path: [p=32j+h, g, bank]
    b1sb = np.zeros((128, NG, 4, 1), np.float32)
    for g in range(NG):
        for i in range(4):
            for j in range(4):
                q = 16 * g + 4 * i + j
                b1sb[32 * j:32 * j + 32, g, i, 0] = b1[q]
    if has_b1:
        # lin must also carry sum_h w2p*b1 -> fold into bfin?  b1 is per
        # (q,h); lin term sum_h w2p*(u+b1) = lin(u) + sum_h w2p*b1[q]
        bfin = bfin + (w2p * b1).sum(-1)

    return (
        w1bd.astype(f16),
        w2m.astype(f16),
        (-w2m).astype(f16),
        wlin.astype(f16),
        bfin.reshape(128, 1),
        b1sb,
        has_b1,
    )


def _build(bc, has_b1, rep=1):
    """Build the Bass program for one core processing bc batch rows.

    rep>1 wraps the batch loop in a For loop reprocessing the same data
    (benchmarking only: amplifies kernel time over ~90ms axon dispatch)."""
    nc = bass.Bass()

    x_d = nc.dram_tensor("x", [bc, C], F16, kind="ExternalInput")
    w1_d = nc.dram_tensor("w1bd", [128, NG, 4, 128], F16, kind="ExternalInput")
    w2_d = nc.dram_tensor("w2m", [128, NG, 4, 32], F16, kind="ExternalInput")
    w2n_d = nc.dram_tensor("w2n", [128, NG, 4, 32], F16, kind="ExternalInput")
    wl_d = nc.dram_tensor("wlin", [128, NG, 32], F16, kind="ExternalInput")
    bf_d = nc.dram_tensor("bfv", [128, 1], F32, kind="ExternalInput")
    b1_d = nc.dram_tensor("b1sb", [128, NG, 4, 1], F32, kind="ExternalInput")
    out_d = nc.dram_tensor("out", [128, bc], F32, kind="ExternalOutput")

    n_tiles = bc // NB
    Exp = mybir.ActivationFunctionType.Exp
    Ident = mybir.ActivationFunctionType.Identity

    with tile.TileContext(nc) as tc:
        _split_waits(tc)
        with (
            tc.tile_pool(name="singles", bufs=1) as singles,
            tc.tile_pool(name="xt", bufs=int(_os.environ.get("XTBUFS", "6"))) as xt_pool,
            tc.tile_pool(name="vv", bufs=int(_os.environ.get("MIDBUFS", "4"))) as vv_pool,
            tc.tile_pool(name="tt", bufs=int(_os.environ.get("MIDBUFS", "4"))) as tt_pool,
            tc.tile_pool(name="ob", bufs=2) as ob_pool,
            tc.tile_pool(name="ps_u", bufs=3, space="PSUM") as ps_u,
            tc.tile_pool(name="ps_o", bufs=2, space="PSUM") as ps_o,
        ):
            w1t = singles.tile([128, NG, 4, 128], F16)
            w2t = singles.tile([128, NG, 4, 32], F16)
            w2nt = singles.tile([128, NG, 4, 32], F16)
            wlt = singles.tile([128, NG, 32], F16)
            bfvt = singles.tile([128, 1], F32)
            b1sb = singles.tile([128, NG, 4, 1], F32)
            wdum = singles.tile([128, 8], F32)

            nc.sync.dma_start(w1t[:], w1_d[:])
            nc.sync.dma_start(w2t[:], w2_d[:])
            nc.sync.dma_start(w2nt[:], w2n_d[:])
            nc.sync.dma_start(wlt[:], wl_d[:])
            nc.sync.dma_start(bfvt[:], bf_d[:])
            nc.sync.dma_start(b1sb[:], b1_d[:])

            # Warmup: make each engine observe each one-time producer once so
            # steady-state instructions need at most one semaphore wait.
            pdum = ps_u.tile([128, 2, NB], F32, tag="u")
            nc.tensor.matmul(pdum[0:128, 0, 0:32], w1t[:, 0, 0, :],
                             w2t[:, 0, 0, :], start=True, stop=True)
            nc.tensor.matmul(pdum[0:32, 1, 0:32], w2nt[:, 0, 0, :],
                             wlt[:, 0, :], start=True, stop=True)
            nc.scalar.activation(wdum[:, 0:1], b1sb[:, 0, 0, :], Exp)
            nc.scalar.activation(wdum[:, 1:2], bfvt[:], Ident, bias=bfvt[:])
            nc.vector.tensor_scalar_min(wdum[:, 2:3], b1sb[:, 0, 0, :], 1.0)
            if "p" in VMIN_ENG or OUT_ENG == "p":
                nc.gpsimd.tensor_scalar_min(wdum[:, 3:4], wdum[:, 2:3], 1.0)

            loop_cm = tc.For_i(0, rep, 1) if rep > 1 else contextlib.nullcontext()
            with loop_cm:
              for n in range(n_tiles):
                outq = ps_o.tile([128, NB], F32, tag="outq")
                vs = {}
                ts = {}

                def d2_chain(p):
                    base = 32 * p
                    mms = []
                    for gg in (2 * p, 2 * p + 1):
                        mms.append((wlt[:, gg, :], None, gg))
                    for gg in (2 * p, 2 * p + 1):
                        for i in range(4):
                            mms.append((w2nt[:, gg, i, :], vs[gg], i))
                        for i in range(4):
                            mms.append((w2t[:, gg, i, :], ts[gg], i))
                    for k, (lhs, mid, i) in enumerate(mms):
                        if mid is None:
                            rhs = xts[i]
                        else:
                            rhs = mid[:, i, :]
                        nc.tensor.matmul(
                            outq[base:base + 32, :], lhs, rhs,
                            start=(k == 0), stop=(k == len(mms) - 1),
                            tile_position=(0, base),
                        )
                    del vs[2 * p], vs[2 * p + 1]
                    del ts[2 * p], ts[2 * p + 1]

                xts = {}
                for g in range(NG):
                    xt = xt_pool.tile([128, NB], F16, tag="xt")
                    nc.sync.dma_start_transpose(
                        xt[:], x_d[NB * n:NB * (n + 1), 128 * g:128 * (g + 1)]
                    )
                    xts[g] = xt

                    ua = ps_u.tile([128, 2, NB], F32, tag="u")
                    ub = ps_u.tile([128, 2, NB], F32, tag="u")
                    for i in range(4):
                        uh = ua if i < 2 else ub
                        nc.tensor.matmul(
                            uh[:, i % 2, :], w1t[:, g, i, :], xt[:],
                            start=True, stop=True,
                        )
                    if has_b1:
                        for i in range(4):
                            uh = ua if i < 2 else ub
                            nc.vector.tensor_scalar_add(
                                uh[:, i % 2, :], uh[:, i % 2, :],
                                b1sb[:, g, i, :])

                    v = vv_pool.tile([128, 4, NB], F16, tag="v")
                    veng = nc.vector if VMIN_ENG[g] == "v" else nc.gpsimd
                    veng.tensor_scalar_min(v[:, 0:2, :], ua[:], 0.0)
                    veng.tensor_scalar_min(v[:, 2:4, :], ub[:], 0.0)
                    vs[g] = v

                    t = tt_pool.tile([128, 4, NB], F16, tag="t")
                    nc.scalar.activation(t[:], v[:], Exp)
                    ts[g] = t

                    # dense2 deferred by one group for pipelining
                    if g >= 3 and g % 2 == 1:
                        d2_chain((g - 3) // 2)
                d2_chain(3)

                ob = ob_pool.tile([128, NB], F32, tag="ob")
                if OUT_ENG == "a":
                    nc.scalar.activation(ob[:], outq[:], Ident, bias=bfvt[:])
                else:
                    oeng = nc.vector if OUT_ENG == "v" else nc.gpsimd
                    oeng.tensor_scalar_add(ob[:], outq[:], bfvt[:])
                nc.sync.dma_start(out_d[:, NB * n:NB * (n + 1)], ob[:])

    return nc


_CACHE = {}


def _get_nc(bc, has_b1, rep=1):
    key = (bc, has_b1, rep, VMIN_ENG, OUT_ENG,
           _os.environ.get("XTBUFS", "6"), _os.environ.get("MIDBUFS", "4"))
    if key not in _CACHE:
        _CACHE[key] = _build(bc, has_b1, rep)
    return _CACHE[key]


def kernel(x, W1, b1, gamma, beta, mov_mean, mov_var, W2, b2):
    x = np.asarray(x, np.float32).reshape(-1, C)
    B = x.shape[0]
    w1bd, w2m, w2n, wlin, bfv, b1sb, has_b1 = _host_pack(
        W1, b1, gamma, beta, mov_mean, mov_var, W2, b2
    )
    x16 = x.astype(np.float16)

    bc = B // NCORES
    rep = int(_os.environ.get("KREP", "1"))
    nc = _get_nc(bc, has_b1, rep)

    in_maps = [
        {
            "x": np.ascontiguousarray(x16[i * bc:(i + 1) * bc]),
            "w1bd": w1bd,
            "w2m": w2m,
            "w2n": w2n,
            "wlin": wlin,
            "bfv": bfv,
            "b1sb": b1sb,
        }
        for i in range(NCORES)
    ]
    res = run_bass_kernel_spmd(nc, in_maps, list(range(NCORES)))
    kernel._last_results = res
    out = np.concatenate(
        [res.results[i]["out"].T for i in range(NCORES)], axis=0
    )
    return np.ascontiguousarray(out).astype(np.float32)
